# revision 14
# baseline (speedup 1.0000x reference)
"""Tensor-parallel GQA attention (RoPE + causal softmax + out-proj) on 8 NeuronCores.

Sharding: heads. Each core owns 4 q-heads + 1 kv-head (wq/wk/wv output rows,
wo input cols). x is replicated; each core computes a partial output
(its heads' contribution through wo) and the host sums the 8 partials.

Per-core dataflow (bf16 matmul operands, fp32 accumulation):
  xT [d, t] --(wfT)--> QT [256, t], KT2 [128(dup), t], VT --(PE transpose)--> Vaug [t, 64|1s]
  RoPE on QT/KT via pair-swap matmul + cos/sin combines.
  S^T [k, q] = KT-chunk.T @ QT-chunk (head parity in 64-partition halves)
  P^T = exp(S^T/8 - 8)  (ACT on 4-bank PSUM spans, causal mask via one affine_select)
  [O^T; denom] = Vaug.T @ P^T  (ones column yields the softmax denominator)
  attn_outT = O^T * 1/denom  (recip_approx + gpsimd partition_broadcast)
  partial = attn_outT.T @ woT  -> DRAM
"""
import numpy as np
import ml_dtypes

import concourse.bass as bass
import concourse.mybir as mybir
from concourse import bacc
from concourse.tile import TileContext
from concourse.bass_utils import run_bass_kernel_spmd
from concourse.masks import make_identity

F32 = mybir.dt.float32
BF16 = mybir.dt.bfloat16
NPBF16 = ml_dtypes.bfloat16

DIM = 2048
N_HEADS = 32
N_KV_HEADS = 8
HEAD_DIM = 64
BSZ = 2
SEQ = 2048
NCORES = 8
HQ = N_HEADS // NCORES          # 4 local q heads
DQ = HQ * HEAD_DIM              # 256
BS = BSZ * SEQ                  # 4096 tokens
NT = SEQ // 512                 # 4 t-chunks (512) per batch
EXP_SHIFT = -8.0
SM_SCALE = 0.125                # 1/sqrt(64)

_BUILT = {}
_last_in_maps = None


def _build(causal: bool):
    nc = bacc.Bacc(None, target_bir_lowering=False, debug=False)

    xT = nc.declare_dram_parameter("xT", [DIM, BS], BF16, isOutput=False)
    wf = nc.declare_dram_parameter("wf", [DIM, DQ + 2 * HEAD_DIM], BF16, isOutput=False)
    woT = nc.declare_dram_parameter("woT", [DQ, DIM], BF16, isOutput=False)
    cosE = nc.declare_dram_parameter("cosE", [128, SEQ], F32, isOutput=False)
    sinE = nc.declare_dram_parameter("sinE", [128, SEQ], F32, isOutput=False)
    pswap = nc.declare_dram_parameter("pswap", [128, 128], BF16, isOutput=False)
    out_p = nc.declare_dram_parameter("out_p", [BS, DIM], F32, isOutput=True)

    with TileContext(nc) as tc:
        with (
            tc.tile_pool(name="wts", bufs=1) as wts,
            tc.tile_pool(name="big", bufs=1) as big,
            tc.tile_pool(name="xs", bufs=2) as xs,
            tc.tile_pool(name="tmp", bufs=3) as tmp,
            tc.tile_pool(name="pts", bufs=4) as pts,
            tc.tile_pool(name="stg", bufs=3) as stg,
            tc.tile_pool(name="psS", bufs=2, space="PSUM") as psS,
            tc.tile_pool(name="psP", bufs=2, space="PSUM") as psP,
            tc.tile_pool(name="psV", bufs=2, space="PSUM") as psV,
        ):
            # ---- resident weights / constants ----
            wf_sb = wts.tile([128, DIM // 128, DQ + 2 * HEAD_DIM], BF16, tag="wf")
            nc.sync.dma_start(wf_sb[:], wf[:].rearrange("(dc p) e -> p dc e", p=128))
            wo_sb = wts.tile([128, DQ // 128, DIM], BF16, tag="wo")
            nc.sync.dma_start(wo_sb[:], woT[:].rearrange("(ec p) d -> p ec d", p=128))
            cos_sb = wts.tile([128, SEQ], F32, tag="cos")
            nc.sync.dma_start(cos_sb[:], cosE[:])
            sin_sb = wts.tile([128, SEQ], F32, tag="sin")
            nc.sync.dma_start(sin_sb[:], sinE[:])
            psw_sb = wts.tile([128, 128], BF16, tag="psw")
            nc.sync.dma_start(psw_sb[:], pswap[:])
            idt32 = wts.tile([128, 128], F32, tag="idt32")
            make_identity(nc, idt32[:])
            idt = wts.tile([128, 128], BF16, tag="idt")
            nc.vector.tensor_copy(idt[:], idt32[:])

            ones_c = wts.tile([128, 1], F32, tag="ones")
            nc.vector.memset(ones_c[:], 1.0)
            bias_c = wts.tile([128, 1], F32, tag="bias")
            nc.vector.memset(bias_c[:], EXP_SHIFT)

            # ---- per-batch activations (bf16), reused across the 2 batches ----
            QTd = [big.tile([128, SEQ], BF16, tag=f"qtd{i}", name=f"qtd{i}")
                   for i in range(4)]
            KT2 = big.tile([128, SEQ], BF16, tag="kt2")
            Vaug = big.tile([128, SEQ // 128, HEAD_DIM + 1], BF16, tag="vaug")
            attnT = [big.tile([128, SEQ], BF16, tag=f"at{i}", name=f"at{i}")
                     for i in range(2)]
            nc.vector.tensor_copy(
                Vaug[:, :, HEAD_DIM : HEAD_DIM + 1],
                ones_c[:, 0:1, None].to_broadcast((128, SEQ // 128, 1)),
            )

            NKC = DIM // 128  # 16 contraction chunks

            def proj(b, tcn):
                """Project t-chunk (512 tokens), RoPE, fill QT/KT2/Vaug."""
                t0 = b * SEQ + tcn * 512   # global (xT read)
                s0 = tcn * 512             # local within batch
                xtile = xs.tile([128, NKC, 512], BF16, tag="xt")
                nc.sync.dma_start(
                    xtile[:],
                    xT[:, t0 : t0 + 512].rearrange("(dc p) t -> p dc t", p=128),
                )
                # --- KV chain first (feeds V transposes + K rope early) ---
                pKV = psP.tile([128, 512], F32, tag="pp", name="pkv")
                for dc in range(NKC):
                    nc.tensor.matmul(
                        pKV[:], wf_sb[:, dc, 256:384], xtile[:, dc, :],
                        start=dc == 0, stop=dc == NKC - 1,
                    )
                # K rope
                kraw = tmp.tile([128, 512], BF16, tag="qraw")
                nc.vector.tensor_copy(kraw[0:64, :], pKV[0:64, :])
                # V -> Vaug via PE transpose (V^T at psum partitions 64:128)
                vt_sb = tmp.tile([128, 512], BF16, tag="vt")
                nc.vector.tensor_copy(vt_sb[0:64, :], pKV[64:128, :])
                ksw_ps = psS.tile([128, 2, 512], F32, tag="sq")
                nc.tensor.matmul(
                    ksw_ps[0:64, 0, :], psw_sb[0:64, 0:64], kraw[0:64, :],
                    start=True, stop=True,
                )
                t1 = tmp.tile([128, 512], F32, tag="t1")
                nc.vector.tensor_tensor(
                    t1[0:64, :], kraw[0:64, :], cos_sb[0:64, s0 : s0 + 512],
                    mybir.AluOpType.mult,
                )
                t2 = tmp.tile([128, 512], F32, tag="t2")
                nc.vector.tensor_tensor(
                    t2[0:64, :], ksw_ps[0:64, 0, :], sin_sb[0:64, s0 : s0 + 512],
                    mybir.AluOpType.mult,
                )
                nc.vector.tensor_tensor(
                    KT2[0:64, s0 : s0 + 512], t1[0:64, :], t2[0:64, :],
                    mybir.AluOpType.add,
                )
                # duplicate rope'd K into partitions 64:128 (for per-parity scores)
                nc.sync.dma_start(KT2[64:128, s0 : s0 + 512], KT2[0:64, s0 : s0 + 512])
                for i in range(4):
                    kig = s0 // 128 + i
                    vtp = psS.tile([128, HEAD_DIM], BF16, tag="sq")
                    nc.tensor.transpose(
                        vtp[:, :], vt_sb[0:64, i * 128 : (i + 1) * 128], idt[0:64, 0:64]
                    )
                    nc.vector.tensor_copy(Vaug[:, kig, 0:HEAD_DIM], vtp[:])
                # --- Q chains ---
                for ch in range(2):
                    pQ = psP.tile([128, 512], F32, tag="pp", name="pq")
                    for dc in range(NKC):
                        nc.tensor.matmul(
                            pQ[:], wf_sb[:, dc, ch * 128 : (ch + 1) * 128],
                            xtile[:, dc, :],
                            start=dc == 0, stop=dc == NKC - 1,
                        )
                    qraw = tmp.tile([128, 512], BF16, tag="qraw")
                    nc.vector.tensor_copy(qraw[:], pQ[:])
                    psw_ps = psS.tile([128, 2, 512], F32, tag="sq")
                    nc.tensor.matmul(
                        psw_ps[:, 0, :], psw_sb[:], qraw[:], start=True, stop=True
                    )
                    t1 = tmp.tile([128, 512], F32, tag="t1")
                    nc.vector.tensor_tensor(
                        t1[:], qraw[:], cos_sb[:, s0 : s0 + 512], mybir.AluOpType.mult
                    )
                    t2 = tmp.tile([128, 512], F32, tag="t2")
                    nc.vector.tensor_tensor(
                        t2[:], psw_ps[:, 0, :], sin_sb[:, s0 : s0 + 512],
                        mybir.AluOpType.mult,
                    )
                    qd0, qd1 = QTd[2 * ch], QTd[2 * ch + 1]
                    nc.vector.tensor_tensor(
                        qd1[:, s0 : s0 + 512], t1[:], t2[:], mybir.AluOpType.add
                    )
                    # qd1 now holds [head-even | head-odd]; spread into per-head dups
                    nc.sync.dma_start(qd0[0:64, s0 : s0 + 512], qd1[0:64, s0 : s0 + 512])
                    nc.sync.dma_start(qd0[64:128, s0 : s0 + 512], qd1[0:64, s0 : s0 + 512])
                    nc.sync.dma_start(qd1[0:64, s0 : s0 + 512], qd1[64:128, s0 : s0 + 512])

            def attention(b, qj):
                """All 4 local heads for q-chunk qj (512 q, local within batch)."""
                t0 = qj * 512
                npr = 2 * (qj + 1) if causal else 2 * NT   # k-pairs (2 k-tiles each)
                kmax = 2 * npr
                for ch in range(2):           # head pair (2ch, 2ch+1)
                    ppv = [psV.tile([128, 512], F32, tag="pv", name=f"ppv{par}")
                           for par in range(2)]
                    for kq in range(npr):
                        for par in range(2):
                            h = 2 * ch + par
                            sq_ps = psS.tile([128, 2, 512], F32, tag="sq")
                            for i in range(2):
                                # row-packed: k-tile i uses PE rows 64i..64i+64
                                kl = kq * 2 + i
                                nc.tensor.matmul(
                                    sq_ps[:, i, :],
                                    KT2[64 * i : 64 * i + 64, kl * 128 : (kl + 1) * 128],
                                    QTd[h][64 * i : 64 * i + 64, t0 : t0 + 512],
                                    start=True, stop=True,
                                )
                            pt = pts.tile([128, 2, 512], BF16, tag="pt")
                            nc.scalar.activation(
                                pt[:], sq_ps[:], mybir.ActivationFunctionType.Exp,
                                bias=bias_c[:], scale=SM_SCALE,
                            )
                            if causal and kq // 2 == qj:
                                # diagonal pair: zero where k > q
                                nc.gpsimd.affine_select(
                                    out=pt[:], in_=pt[:],
                                    compare_op=mybir.AluOpType.is_ge,
                                    fill=0.0,
                                    base=512 * qj - 128 * (2 * kq),
                                    pattern=[[-128, 2], [1, 512]],
                                    channel_multiplier=-1,
                                )
                            for i in range(2):
                                kl = kq * 2 + i
                                nc.tensor.matmul(
                                    ppv[par][0:65, :], Vaug[:, kl, :], pt[:, i, :],
                                    start=(kl == 0), stop=(kl == kmax - 1),
                                    skip_group_check=True,
                                )
                    for par in range(2):
                        base = 64 * par
                        # denominator -> reciprocal -> broadcast -> normalize+evac
                        drow = stg.tile([1, 512], F32, tag="drow")
                        nc.vector.tensor_copy(drow[0:1, :], ppv[par][64:65, :])
                        drec = stg.tile([1, 512], F32, tag="drec")
                        nc.vector.reciprocal_approx_fast(drec[0:1, :], drow[0:1, :])
                        rB = stg.tile([64, 512], F32, tag="rB")
                        nc.gpsimd.partition_broadcast(rB[:], drec[0:1, :])
                        nc.vector.tensor_tensor(
                            attnT[ch][base : base + 64, t0 : t0 + 512],
                            ppv[par][0:64, :], rB[:], mybir.AluOpType.mult,
                        )

            def out_proj(b, tcn):
                for tt in range(4):
                    tl = tcn * 512 + tt * 128      # local (attnT read)
                    ta = b * SEQ + tl              # global (out write)
                    for dp in range(DIM // 1024):  # pairs of 512-wide dd chunks
                        wo_ps = psS.tile([128, 2, 512], F32, tag="sq")
                        for j in range(2):
                            dd = dp * 2 + j
                            for ec in range(2):
                                nc.tensor.matmul(
                                    wo_ps[:, j, :],
                                    attnT[ec][:, tl : tl + 128],
                                    wo_sb[:, ec, dd * 512 : (dd + 1) * 512],
                                    start=(ec == 0), stop=(ec == 1),
                                )
                        so = stg.tile([128, 1024], F32, tag="so")
                        nc.vector.tensor_copy(so[:], wo_ps[:].rearrange("p a b -> p (a b)"))
                        nc.sync.dma_start(
                            out_p[ta : ta + 128, dp * 1024 : (dp + 1) * 1024], so[:]
                        )

            for b in range(BSZ):
                for tcn in range(NT):
                    proj(b, tcn)
                    if causal:
                        attention(b, tcn)
                        out_proj(b, tcn)
                if not causal:
                    for qj in range(NT):
                        attention(b, qj)
                        out_proj(b, qj)

    nc.compile()
    return nc


def _get_nc(causal: bool):
    if causal not in _BUILT:
        _BUILT[causal] = _build(causal)
    return _BUILT[causal]


def kernel(x, freqs_cos, freqs_sin, mask, wq, wk, wv, wo):
    x = np.asarray(x, dtype=np.float32)
    freqs_cos = np.asarray(freqs_cos, dtype=np.float32)
    freqs_sin = np.asarray(freqs_sin, dtype=np.float32)
    mask = np.asarray(mask, dtype=np.float32)
    wq = np.asarray(wq, dtype=np.float32)
    wk = np.asarray(wk, dtype=np.float32)
    wv = np.asarray(wv, dtype=np.float32)
    wo = np.asarray(wo, dtype=np.float32)

    causal = bool(mask.reshape(SEQ, SEQ)[0, -1] < -1e30)
    nc = _get_nc(causal)

    xT = np.ascontiguousarray(x.reshape(BS, DIM).T.astype(NPBF16))

    # RoPE row tables: row p (within 64) = pair (p%64)//2, sign by parity
    j = (np.arange(128) % 64) // 2
    par = np.arange(128) % 2
    cosE = np.ascontiguousarray(freqs_cos[:, j].T)            # [128, SEQ]
    sgn = np.where(par == 1, 1.0, -1.0).astype(np.float32)
    sinE = np.ascontiguousarray(freqs_sin[:, j].T * sgn[:, None])
    pswap = np.zeros((128, 128), dtype=NPBF16)
    idx = np.arange(128)
    pswap[idx, idx ^ 1] = 1.0

    in_maps = []
    for c in range(NCORES):
        wq_c = wq[c * DQ : (c + 1) * DQ]                       # [256, D]
        wk_c = wk[c * HEAD_DIM : (c + 1) * HEAD_DIM]           # [64, D]
        wv_c = wv[c * HEAD_DIM : (c + 1) * HEAD_DIM]
        wf_c = np.ascontiguousarray(
            np.concatenate([wq_c, wk_c, wv_c], axis=0).T.astype(NPBF16))  # [D, 384]
        woT_c = np.ascontiguousarray(
            wo[:, c * DQ : (c + 1) * DQ].T.astype(NPBF16))     # [256, D]
        in_maps.append({
            "xT": xT, "wf": wf_c, "woT": woT_c,
            "cosE": cosE, "sinE": sinE, "pswap": pswap,
        })

    global _last_in_maps
    _last_in_maps = in_maps
    res = run_bass_kernel_spmd(nc, in_maps, core_ids=list(range(NCORES)))
    out = res.results[0]["out_p"].astype(np.float64)
    for c in range(1, NCORES):
        out += res.results[c]["out_p"]
    return out.astype(np.float32).reshape(BSZ, SEQ, DIM)


# revision 15
# speedup vs baseline: 1.0771x; 1.0771x over previous
"""Tensor-parallel GQA attention (RoPE + causal softmax + out-proj) on 8 NeuronCores.

Sharding: heads. Each core owns 4 q-heads + 1 kv-head (wq/wk/wv output rows,
wo input cols). x is replicated; each core computes a partial output
(its heads' contribution through wo) and the host sums the 8 partials.

Per-core dataflow (bf16 matmul operands, fp32 accumulation):
  xT [d, t] --(wfT)--> QT [256, t], KT2 [128(dup), t], VT --(PE transpose)--> Vaug [t, 64|1s]
  RoPE on QT/KT via pair-swap matmul + cos/sin combines.
  S^T [k, q] = KT-chunk.T @ QT-chunk (head parity in 64-partition halves)
  P^T = exp(S^T/8 - 8)  (ACT on 4-bank PSUM spans, causal mask via one affine_select)
  [O^T; denom] = Vaug.T @ P^T  (ones column yields the softmax denominator)
  attn_outT = O^T * 1/denom  (recip_approx + gpsimd partition_broadcast)
  partial = attn_outT.T @ woT  -> DRAM
"""
import numpy as np
import ml_dtypes

import concourse.bass as bass
import concourse.mybir as mybir
from concourse import bacc
from concourse.tile import TileContext
from concourse.bass_utils import run_bass_kernel_spmd
from concourse.masks import make_identity

F32 = mybir.dt.float32
BF16 = mybir.dt.bfloat16
NPBF16 = ml_dtypes.bfloat16

DIM = 2048
N_HEADS = 32
N_KV_HEADS = 8
HEAD_DIM = 64
BSZ = 2
SEQ = 2048
NCORES = 8
HQ = N_HEADS // NCORES          # 4 local q heads
DQ = HQ * HEAD_DIM              # 256
BS = BSZ * SEQ                  # 4096 tokens
NT = SEQ // 512                 # 4 t-chunks (512) per batch
EXP_SHIFT = -8.0
SM_SCALE = 0.125                # 1/sqrt(64)

_BUILT = {}
_last_in_maps = None


def _build(causal: bool):
    nc = bacc.Bacc(None, target_bir_lowering=False, debug=False)

    xT = nc.declare_dram_parameter("xT", [DIM, BS], BF16, isOutput=False)
    wf = nc.declare_dram_parameter("wf", [DIM, DQ + 2 * HEAD_DIM], BF16, isOutput=False)
    woT = nc.declare_dram_parameter("woT", [DQ, DIM], BF16, isOutput=False)
    cosE = nc.declare_dram_parameter("cosE", [128, SEQ], F32, isOutput=False)
    sinE = nc.declare_dram_parameter("sinE", [128, SEQ], F32, isOutput=False)
    pswap = nc.declare_dram_parameter("pswap", [128, 128], BF16, isOutput=False)
    out_p = nc.declare_dram_parameter("out_p", [BS, DIM], F32, isOutput=True)

    with TileContext(nc) as tc:
        with (
            tc.tile_pool(name="wts", bufs=1) as wts,
            tc.tile_pool(name="big", bufs=1) as big,
            tc.tile_pool(name="xs", bufs=2) as xs,
            tc.tile_pool(name="tmp", bufs=3) as tmp,
            tc.tile_pool(name="pts", bufs=4) as pts,
            tc.tile_pool(name="stg", bufs=3) as stg,
            tc.tile_pool(name="psS", bufs=2, space="PSUM") as psS,
            tc.tile_pool(name="psP", bufs=2, space="PSUM") as psP,
            tc.tile_pool(name="psV", bufs=2, space="PSUM") as psV,
        ):
            # ---- resident weights / constants ----
            wf_sb = wts.tile([128, DIM // 128, DQ + 2 * HEAD_DIM], BF16, tag="wf")
            nc.sync.dma_start(wf_sb[:], wf[:].rearrange("(dc p) e -> p dc e", p=128))
            wo_sb = wts.tile([128, DQ // 128, DIM], BF16, tag="wo")
            nc.sync.dma_start(wo_sb[:], woT[:].rearrange("(ec p) d -> p ec d", p=128))
            cos_sb = wts.tile([128, SEQ], F32, tag="cos")
            nc.sync.dma_start(cos_sb[:], cosE[:])
            sin_sb = wts.tile([128, SEQ], F32, tag="sin")
            nc.sync.dma_start(sin_sb[:], sinE[:])
            psw_sb = wts.tile([128, 128], BF16, tag="psw")
            nc.sync.dma_start(psw_sb[:], pswap[:])
            idt32 = wts.tile([128, 128], F32, tag="idt32")
            make_identity(nc, idt32[:])
            idt = wts.tile([128, 128], BF16, tag="idt")
            nc.vector.tensor_copy(idt[:], idt32[:])

            ones_c = wts.tile([128, 1], F32, tag="ones")
            nc.vector.memset(ones_c[:], 1.0)
            bias_c = wts.tile([128, 1], F32, tag="bias")
            nc.vector.memset(bias_c[:], EXP_SHIFT)

            # ---- per-batch activations (bf16), reused across the 2 batches ----
            QTd = [big.tile([128, SEQ], BF16, tag=f"qtd{i}", name=f"qtd{i}")
                   for i in range(4)]
            KT2 = big.tile([128, SEQ], BF16, tag="kt2")
            Vaug = big.tile([128, SEQ // 128, HEAD_DIM + 1], BF16, tag="vaug")
            attnT = [big.tile([128, SEQ], BF16, tag=f"at{i}", name=f"at{i}")
                     for i in range(2)]
            nc.vector.tensor_copy(
                Vaug[:, :, HEAD_DIM : HEAD_DIM + 1],
                ones_c[:, 0:1, None].to_broadcast((128, SEQ // 128, 1)),
            )

            NKC = DIM // 128  # 16 contraction chunks

            def proj(b, tcn):
                """Project t-chunk (512 tokens), RoPE, fill QT/KT2/Vaug."""
                t0 = b * SEQ + tcn * 512   # global (xT read)
                s0 = tcn * 512             # local within batch
                xtile = xs.tile([128, NKC, 512], BF16, tag="xt")
                nc.sync.dma_start(
                    xtile[:],
                    xT[:, t0 : t0 + 512].rearrange("(dc p) t -> p dc t", p=128),
                )
                # --- KV chain first (feeds V transposes + K rope early) ---
                pKV = psP.tile([128, 512], F32, tag="pp", name="pkv")
                for dc in range(NKC):
                    nc.tensor.matmul(
                        pKV[:], wf_sb[:, dc, 256:384], xtile[:, dc, :],
                        start=dc == 0, stop=dc == NKC - 1,
                    )
                # K rope
                kraw = tmp.tile([128, 512], BF16, tag="qraw")
                nc.vector.tensor_copy(kraw[0:64, :], pKV[0:64, :])
                # V -> Vaug via PE transpose (V^T at psum partitions 64:128)
                vt_sb = tmp.tile([128, 512], BF16, tag="vt")
                nc.vector.tensor_copy(vt_sb[0:64, :], pKV[64:128, :])
                ksw_ps = psS.tile([128, 2, 512], F32, tag="sq")
                nc.tensor.matmul(
                    ksw_ps[0:64, 0, :], psw_sb[0:64, 0:64], kraw[0:64, :],
                    start=True, stop=True,
                )
                t1 = tmp.tile([128, 512], F32, tag="t1")
                nc.vector.tensor_tensor(
                    t1[0:64, :], kraw[0:64, :], cos_sb[0:64, s0 : s0 + 512],
                    mybir.AluOpType.mult,
                )
                t2 = tmp.tile([128, 512], F32, tag="t2")
                nc.vector.tensor_tensor(
                    t2[0:64, :], ksw_ps[0:64, 0, :], sin_sb[0:64, s0 : s0 + 512],
                    mybir.AluOpType.mult,
                )
                nc.vector.tensor_tensor(
                    KT2[0:64, s0 : s0 + 512], t1[0:64, :], t2[0:64, :],
                    mybir.AluOpType.add,
                )
                # duplicate rope'd K into partitions 64:128 (for per-parity scores)
                nc.sync.dma_start(KT2[64:128, s0 : s0 + 512], KT2[0:64, s0 : s0 + 512])
                for i in range(4):
                    kig = s0 // 128 + i
                    vtp = psS.tile([128, HEAD_DIM], BF16, tag="sq")
                    nc.tensor.transpose(
                        vtp[:, :], vt_sb[0:64, i * 128 : (i + 1) * 128], idt[0:64, 0:64]
                    )
                    nc.vector.tensor_copy(Vaug[:, kig, 0:HEAD_DIM], vtp[:])
                # --- Q chains ---
                for ch in range(2):
                    pQ = psP.tile([128, 512], F32, tag="pp", name="pq")
                    for dc in range(NKC):
                        nc.tensor.matmul(
                            pQ[:], wf_sb[:, dc, ch * 128 : (ch + 1) * 128],
                            xtile[:, dc, :],
                            start=dc == 0, stop=dc == NKC - 1,
                        )
                    qraw = tmp.tile([128, 512], BF16, tag="qraw")
                    nc.vector.tensor_copy(qraw[:], pQ[:])
                    psw_ps = psS.tile([128, 2, 512], F32, tag="sq")
                    nc.tensor.matmul(
                        psw_ps[:, 0, :], psw_sb[:], qraw[:], start=True, stop=True
                    )
                    t1 = tmp.tile([128, 512], F32, tag="t1")
                    nc.vector.tensor_tensor(
                        t1[:], qraw[:], cos_sb[:, s0 : s0 + 512], mybir.AluOpType.mult
                    )
                    t2 = tmp.tile([128, 512], F32, tag="t2")
                    nc.vector.tensor_tensor(
                        t2[:], psw_ps[:, 0, :], sin_sb[:, s0 : s0 + 512],
                        mybir.AluOpType.mult,
                    )
                    qd0, qd1 = QTd[2 * ch], QTd[2 * ch + 1]
                    nc.vector.tensor_tensor(
                        qd1[:, s0 : s0 + 512], t1[:], t2[:], mybir.AluOpType.add
                    )
                    # qd1 now holds [head-even | head-odd]; spread into per-head dups
                    nc.sync.dma_start(qd0[0:64, s0 : s0 + 512], qd1[0:64, s0 : s0 + 512])
                    nc.sync.dma_start(qd0[64:128, s0 : s0 + 512], qd1[0:64, s0 : s0 + 512])
                    nc.sync.dma_start(qd1[0:64, s0 : s0 + 512], qd1[64:128, s0 : s0 + 512])

            def attention(b, qj, filler=None):
                """All 4 local heads for q-chunk qj; filler() emits PE work
                between dependency-stalled attention slots."""
                t0 = qj * 512
                npr = 2 * (qj + 1) if causal else 2 * NT   # k-pairs (2 k-tiles each)
                kmax = 2 * npr

                def emit_scores(ch, kq, par, ptl):
                    h = 2 * ch + par
                    sq_ps = psS.tile([128, 2, 512], F32, tag="sq")
                    for i in range(2):
                        kl = kq * 2 + i
                        nc.tensor.matmul(
                            sq_ps[:, i, :],
                            KT2[64 * i : 64 * i + 64, kl * 128 : (kl + 1) * 128],
                            QTd[h][64 * i : 64 * i + 64, t0 : t0 + 512],
                            start=True, stop=True,
                        )
                    nc.scalar.activation(
                        ptl[:], sq_ps[:], mybir.ActivationFunctionType.Exp,
                        bias=bias_c[:], scale=SM_SCALE,
                    )
                    if causal and kq // 2 == qj:
                        nc.gpsimd.affine_select(
                            out=ptl[:], in_=ptl[:],
                            compare_op=mybir.AluOpType.is_ge,
                            fill=0.0,
                            base=512 * qj - 128 * (2 * kq),
                            pattern=[[-128, 2], [1, 512]],
                            channel_multiplier=-1,
                        )

                def emit_pv(ppv_t, kq, ptl):
                    for i in range(2):
                        kl = kq * 2 + i
                        nc.tensor.matmul(
                            ppv_t[0:65, :], Vaug[:, kl, :], ptl[:, i, :],
                            start=(kl == 0), stop=(kl == kmax - 1),
                            skip_group_check=True,
                        )

                for ch in range(2):           # head pair (2ch, 2ch+1)
                    ppv = [psV.tile([128, 512], F32, tag="pv", name=f"ppv{par}")
                           for par in range(2)]
                    prev = None
                    for kq in range(npr):
                        cur = []
                        for par in range(2):
                            ptl = pts.tile([128, 2, 512], BF16, tag="pt")
                            emit_scores(ch, kq, par, ptl)
                            cur.append(ptl)
                        if prev is not None:
                            for par in range(2):
                                emit_pv(ppv[par], kq - 1, prev[par])
                        if filler is not None:
                            filler()
                        prev = cur
                    for par in range(2):
                        emit_pv(ppv[par], npr - 1, prev[par])
                    for par in range(2):
                        base = 64 * par
                        drow = stg.tile([1, 512], F32, tag="drow")
                        nc.vector.tensor_copy(drow[0:1, :], ppv[par][64:65, :])
                        drec = stg.tile([1, 512], F32, tag="drec")
                        nc.vector.reciprocal_approx_fast(drec[0:1, :], drow[0:1, :])
                        rB = stg.tile([64, 512], F32, tag="rB")
                        nc.gpsimd.partition_broadcast(rB[:], drec[0:1, :])
                        nc.vector.tensor_tensor(
                            attnT[ch][base : base + 64, t0 : t0 + 512],
                            ppv[par][0:64, :], rB[:], mybir.AluOpType.mult,
                        )

            def wo_groups(b, tcn):
                """Yield 16 closures, each emitting one (tt, dp) output block."""
                for tt in range(4):
                    for dp in range(DIM // 1024):
                        def emit(tt=tt, dp=dp):
                            tl = tcn * 512 + tt * 128
                            ta = b * SEQ + tl
                            wo_ps = psS.tile([128, 2, 512], F32, tag="sq")
                            for j in range(2):
                                dd = dp * 2 + j
                                for ec in range(2):
                                    nc.tensor.matmul(
                                        wo_ps[:, j, :],
                                        attnT[ec][:, tl : tl + 128],
                                        wo_sb[:, ec, dd * 512 : (dd + 1) * 512],
                                        start=(ec == 0), stop=(ec == 1),
                                    )
                            so = stg.tile([128, 1024], F32, tag="so")
                            nc.vector.tensor_copy(
                                so[:], wo_ps[:].rearrange("p a b -> p (a b)")
                            )
                            nc.sync.dma_start(
                                out_p[ta : ta + 128, dp * 1024 : (dp + 1) * 1024], so[:]
                            )
                        yield emit

            def make_filler(groups):
                def filler():
                    n = next(filler.pace)
                    for _ in range(n):
                        g = next(groups, None)
                        if g is not None:
                            g()
                return filler

            def drain(groups):
                for g in groups:
                    g()

            import itertools

            pending_wo = None
            for b in range(BSZ):
                for tcn in range(NT):
                    proj(b, tcn)
                    if causal:
                        nslots = 2 * 2 * (tcn + 1)   # filler calls this chunk
                        if pending_wo is not None:
                            per = max(1, -(-16 // nslots))
                            fill = make_filler(pending_wo)
                            fill.pace = itertools.repeat(per)
                        else:
                            fill = None
                        attention(b, tcn, filler=fill)
                        if pending_wo is not None:
                            drain(pending_wo)
                        pending_wo = wo_groups(b, tcn)
                if not causal:
                    for qj in range(NT):
                        if pending_wo is not None:
                            fill = make_filler(pending_wo)
                            fill.pace = itertools.repeat(1)
                        else:
                            fill = None
                        attention(b, qj, filler=fill)
                        if pending_wo is not None:
                            drain(pending_wo)
                        pending_wo = wo_groups(b, qj)
            drain(pending_wo)

    nc.compile()
    return nc


def _get_nc(causal: bool):
    if causal not in _BUILT:
        _BUILT[causal] = _build(causal)
    return _BUILT[causal]


def kernel(x, freqs_cos, freqs_sin, mask, wq, wk, wv, wo):
    x = np.asarray(x, dtype=np.float32)
    freqs_cos = np.asarray(freqs_cos, dtype=np.float32)
    freqs_sin = np.asarray(freqs_sin, dtype=np.float32)
    mask = np.asarray(mask, dtype=np.float32)
    wq = np.asarray(wq, dtype=np.float32)
    wk = np.asarray(wk, dtype=np.float32)
    wv = np.asarray(wv, dtype=np.float32)
    wo = np.asarray(wo, dtype=np.float32)

    causal = bool(mask.reshape(SEQ, SEQ)[0, -1] < -1e30)
    nc = _get_nc(causal)

    xT = np.ascontiguousarray(x.reshape(BS, DIM).T.astype(NPBF16))

    # RoPE row tables: row p (within 64) = pair (p%64)//2, sign by parity
    j = (np.arange(128) % 64) // 2
    par = np.arange(128) % 2
    cosE = np.ascontiguousarray(freqs_cos[:, j].T)            # [128, SEQ]
    sgn = np.where(par == 1, 1.0, -1.0).astype(np.float32)
    sinE = np.ascontiguousarray(freqs_sin[:, j].T * sgn[:, None])
    pswap = np.zeros((128, 128), dtype=NPBF16)
    idx = np.arange(128)
    pswap[idx, idx ^ 1] = 1.0

    in_maps = []
    for c in range(NCORES):
        wq_c = wq[c * DQ : (c + 1) * DQ]                       # [256, D]
        wk_c = wk[c * HEAD_DIM : (c + 1) * HEAD_DIM]           # [64, D]
        wv_c = wv[c * HEAD_DIM : (c + 1) * HEAD_DIM]
        wf_c = np.ascontiguousarray(
            np.concatenate([wq_c, wk_c, wv_c], axis=0).T.astype(NPBF16))  # [D, 384]
        woT_c = np.ascontiguousarray(
            wo[:, c * DQ : (c + 1) * DQ].T.astype(NPBF16))     # [256, D]
        in_maps.append({
            "xT": xT, "wf": wf_c, "woT": woT_c,
            "cosE": cosE, "sinE": sinE, "pswap": pswap,
        })

    global _last_in_maps
    _last_in_maps = in_maps
    res = run_bass_kernel_spmd(nc, in_maps, core_ids=list(range(NCORES)))
    out = res.results[0]["out_p"].astype(np.float64)
    for c in range(1, NCORES):
        out += res.results[c]["out_p"]
    return out.astype(np.float32).reshape(BSZ, SEQ, DIM)


# revision 16
# speedup vs baseline: 1.1342x; 1.0530x over previous
"""Tensor-parallel GQA attention (RoPE + causal softmax + out-proj) on 8 NeuronCores.

Sharding: heads. Each core owns 4 q-heads + 1 kv-head (wq/wk/wv output rows,
wo input cols). x is replicated; each core computes a partial output
(its heads' contribution through wo) and the host sums the 8 partials.

Per-core dataflow (bf16 matmul operands, fp32 accumulation):
  xT [d, t] --(wfT)--> QT [256, t], KT2 [128(dup), t], VT --(PE transpose)--> Vaug [t, 64|1s]
  RoPE on QT/KT via pair-swap matmul + cos/sin combines.
  S^T [k, q] = KT-chunk.T @ QT-chunk (head parity in 64-partition halves)
  P^T = exp(S^T/8 - 8)  (ACT on 4-bank PSUM spans, causal mask via one affine_select)
  [O^T; denom] = Vaug.T @ P^T  (ones column yields the softmax denominator)
  attn_outT = O^T * 1/denom  (recip_approx + gpsimd partition_broadcast)
  partial = attn_outT.T @ woT  -> DRAM
"""
import numpy as np
import ml_dtypes

import concourse.bass as bass
import concourse.mybir as mybir
from concourse import bacc
from concourse.tile import TileContext
from concourse.bass_utils import run_bass_kernel_spmd
from concourse.masks import make_identity

F32 = mybir.dt.float32
BF16 = mybir.dt.bfloat16
NPBF16 = ml_dtypes.bfloat16

DIM = 2048
N_HEADS = 32
N_KV_HEADS = 8
HEAD_DIM = 64
BSZ = 2
SEQ = 2048
NCORES = 8
HQ = N_HEADS // NCORES          # 4 local q heads
DQ = HQ * HEAD_DIM              # 256
BS = BSZ * SEQ                  # 4096 tokens
NT = SEQ // 512                 # 4 t-chunks (512) per batch
EXP_SHIFT = -8.0
SM_SCALE = 0.125                # 1/sqrt(64)

_BUILT = {}
_last_in_maps = None


def _build(causal: bool):
    nc = bacc.Bacc(None, target_bir_lowering=False, debug=False)

    xT = nc.declare_dram_parameter("xT", [DIM, BS], BF16, isOutput=False)
    wf = nc.declare_dram_parameter("wf", [DIM, DQ + 2 * HEAD_DIM], BF16, isOutput=False)
    woT = nc.declare_dram_parameter("woT", [DQ, DIM], BF16, isOutput=False)
    cosE = nc.declare_dram_parameter("cosE", [128, SEQ], F32, isOutput=False)
    sinE = nc.declare_dram_parameter("sinE", [128, SEQ], F32, isOutput=False)
    pswap = nc.declare_dram_parameter("pswap", [128, 128], BF16, isOutput=False)
    out_p = nc.declare_dram_parameter("out_p", [BS, DIM], F32, isOutput=True)

    with TileContext(nc) as tc:
        with (
            tc.tile_pool(name="wts", bufs=1) as wts,
            tc.tile_pool(name="big", bufs=1) as big,
            tc.tile_pool(name="xs", bufs=2) as xs,
            tc.tile_pool(name="tmp", bufs=3) as tmp,
            tc.tile_pool(name="pts", bufs=4) as pts,
            tc.tile_pool(name="stg", bufs=3) as stg,
            tc.tile_pool(name="psS", bufs=2, space="PSUM") as psS,
            tc.tile_pool(name="psP", bufs=2, space="PSUM") as psP,
            tc.tile_pool(name="psV", bufs=2, space="PSUM") as psV,
        ):
            # ---- resident weights / constants ----
            wf_sb = wts.tile([128, DIM // 128, DQ + 2 * HEAD_DIM], BF16, tag="wf")
            nc.sync.dma_start(wf_sb[:], wf[:].rearrange("(dc p) e -> p dc e", p=128))
            cos_sb = wts.tile([128, SEQ], F32, tag="cos")
            nc.sync.dma_start(cos_sb[:], cosE[:])
            sin_sb = wts.tile([128, SEQ], F32, tag="sin")
            nc.sync.dma_start(sin_sb[:], sinE[:])
            wo_sb = wts.tile([128, DQ // 128, DIM], BF16, tag="wo")
            nc.sync.dma_start(wo_sb[:], woT[:].rearrange("(ec p) d -> p ec d", p=128))
            psw_sb = wts.tile([128, 128], BF16, tag="psw")
            nc.sync.dma_start(psw_sb[:], pswap[:])
            idt32 = wts.tile([128, 128], F32, tag="idt32")
            make_identity(nc, idt32[:])
            idt = wts.tile([128, 128], BF16, tag="idt")
            nc.vector.tensor_copy(idt[:], idt32[:])

            ones_c = wts.tile([128, 1], F32, tag="ones")
            nc.vector.memset(ones_c[:], 1.0)
            bias_c = wts.tile([128, 1], F32, tag="bias")
            nc.vector.memset(bias_c[:], EXP_SHIFT)

            # ---- per-batch activations (bf16), reused across the 2 batches ----
            QTd = [big.tile([128, SEQ], BF16, tag=f"qtd{i}", name=f"qtd{i}")
                   for i in range(4)]
            KT2 = big.tile([128, SEQ], BF16, tag="kt2")
            Vaug = big.tile([128, SEQ // 128, HEAD_DIM + 1], BF16, tag="vaug")
            attnT = [big.tile([128, SEQ], BF16, tag=f"at{i}", name=f"at{i}")
                     for i in range(2)]
            nc.vector.tensor_copy(
                Vaug[:, :, HEAD_DIM : HEAD_DIM + 1],
                ones_c[:, 0:1, None].to_broadcast((128, SEQ // 128, 1)),
            )

            NKC = DIM // 128  # 16 contraction chunks

            def proj(b, tcn):
                """Project t-chunk (512 tokens), RoPE, fill QT/KT2/Vaug."""
                t0 = b * SEQ + tcn * 512   # global (xT read)
                s0 = tcn * 512             # local within batch
                xtile = xs.tile([128, NKC, 512], BF16, tag="xt")
                xr = xT[:, t0 : t0 + 512].rearrange("(dc p) t -> p dc t", p=128)
                for q4 in range(4):
                    nc.sync.dma_start(
                        xtile[:, q4 * 4 : (q4 + 1) * 4, :], xr[:, q4 * 4 : (q4 + 1) * 4, :]
                    )
                # --- KV chain first (feeds V transposes + K rope early) ---
                pKV = psP.tile([128, 512], F32, tag="pp", name="pkv")
                for dc in range(NKC):
                    nc.tensor.matmul(
                        pKV[:], wf_sb[:, dc, 256:384], xtile[:, dc, :],
                        start=dc == 0, stop=dc == NKC - 1,
                    )
                # K rope
                kraw = tmp.tile([128, 512], BF16, tag="qraw")
                nc.vector.tensor_copy(kraw[0:64, :], pKV[0:64, :])
                # V -> Vaug via PE transpose (V^T at psum partitions 64:128)
                vt_sb = tmp.tile([128, 512], BF16, tag="vt")
                nc.vector.tensor_copy(vt_sb[0:64, :], pKV[64:128, :])
                ksw_ps = psS.tile([128, 2, 512], F32, tag="sq")
                nc.tensor.matmul(
                    ksw_ps[0:64, 0, :], psw_sb[0:64, 0:64], kraw[0:64, :],
                    start=True, stop=True,
                )
                t1 = tmp.tile([128, 512], F32, tag="t1")
                nc.vector.tensor_tensor(
                    t1[0:64, :], kraw[0:64, :], cos_sb[0:64, s0 : s0 + 512],
                    mybir.AluOpType.mult,
                )
                t2 = tmp.tile([128, 512], F32, tag="t2")
                nc.vector.tensor_tensor(
                    t2[0:64, :], ksw_ps[0:64, 0, :], sin_sb[0:64, s0 : s0 + 512],
                    mybir.AluOpType.mult,
                )
                nc.vector.tensor_tensor(
                    KT2[0:64, s0 : s0 + 512], t1[0:64, :], t2[0:64, :],
                    mybir.AluOpType.add,
                )
                # duplicate rope'd K into partitions 64:128 (for per-parity scores)
                nc.sync.dma_start(KT2[64:128, s0 : s0 + 512], KT2[0:64, s0 : s0 + 512])
                for i in range(4):
                    kig = s0 // 128 + i
                    vtp = psS.tile([128, HEAD_DIM], BF16, tag="sq")
                    nc.tensor.transpose(
                        vtp[:, :], vt_sb[0:64, i * 128 : (i + 1) * 128], idt[0:64, 0:64]
                    )
                    nc.vector.tensor_copy(Vaug[:, kig, 0:HEAD_DIM], vtp[:])
                # --- Q chains ---
                for ch in range(2):
                    pQ = psP.tile([128, 512], F32, tag="pp", name="pq")
                    for dc in range(NKC):
                        nc.tensor.matmul(
                            pQ[:], wf_sb[:, dc, ch * 128 : (ch + 1) * 128],
                            xtile[:, dc, :],
                            start=dc == 0, stop=dc == NKC - 1,
                        )
                    qraw = tmp.tile([128, 512], BF16, tag="qraw")
                    nc.vector.tensor_copy(qraw[:], pQ[:])
                    psw_ps = psS.tile([128, 2, 512], F32, tag="sq")
                    nc.tensor.matmul(
                        psw_ps[:, 0, :], psw_sb[:], qraw[:], start=True, stop=True
                    )
                    t1 = tmp.tile([128, 512], F32, tag="t1")
                    nc.vector.tensor_tensor(
                        t1[:], qraw[:], cos_sb[:, s0 : s0 + 512], mybir.AluOpType.mult
                    )
                    t2 = tmp.tile([128, 512], F32, tag="t2")
                    nc.vector.tensor_tensor(
                        t2[:], psw_ps[:, 0, :], sin_sb[:, s0 : s0 + 512],
                        mybir.AluOpType.mult,
                    )
                    qd0, qd1 = QTd[2 * ch], QTd[2 * ch + 1]
                    nc.vector.tensor_tensor(
                        qd1[:, s0 : s0 + 512], t1[:], t2[:], mybir.AluOpType.add
                    )
                    # qd1 now holds [head-even | head-odd]; spread into per-head dups
                    nc.sync.dma_start(qd0[0:64, s0 : s0 + 512], qd1[0:64, s0 : s0 + 512])
                    nc.sync.dma_start(qd0[64:128, s0 : s0 + 512], qd1[0:64, s0 : s0 + 512])
                    nc.sync.dma_start(qd1[0:64, s0 : s0 + 512], qd1[64:128, s0 : s0 + 512])

            def attention(b, qj, filler=None):
                """All 4 local heads for q-chunk qj; filler() emits PE work
                between dependency-stalled attention slots."""
                t0 = qj * 512
                npr = 2 * (qj + 1) if causal else 2 * NT   # k-pairs (2 k-tiles each)
                kmax = 2 * npr

                def emit_scores(ch, kq, par, ptl):
                    h = 2 * ch + par
                    sq_ps = psS.tile([128, 2, 512], F32, tag="sq")
                    for i in range(2):
                        kl = kq * 2 + i
                        nc.tensor.matmul(
                            sq_ps[:, i, :],
                            KT2[64 * i : 64 * i + 64, kl * 128 : (kl + 1) * 128],
                            QTd[h][64 * i : 64 * i + 64, t0 : t0 + 512],
                            start=True, stop=True,
                        )
                    nc.scalar.activation(
                        ptl[:], sq_ps[:], mybir.ActivationFunctionType.Exp,
                        bias=bias_c[:], scale=SM_SCALE,
                    )
                    if causal and kq // 2 == qj:
                        nc.gpsimd.affine_select(
                            out=ptl[:], in_=ptl[:],
                            compare_op=mybir.AluOpType.is_ge,
                            fill=0.0,
                            base=512 * qj - 128 * (2 * kq),
                            pattern=[[-128, 2], [1, 512]],
                            channel_multiplier=-1,
                        )

                def emit_pv(ppv_t, kq, ptl):
                    for i in range(2):
                        kl = kq * 2 + i
                        nc.tensor.matmul(
                            ppv_t[0:65, :], Vaug[:, kl, :], ptl[:, i, :],
                            start=(kl == 0), stop=(kl == kmax - 1),
                            skip_group_check=True,
                        )

                for ch in range(2):           # head pair (2ch, 2ch+1)
                    ppv = [psV.tile([128, 512], F32, tag="pv", name=f"ppv{par}")
                           for par in range(2)]
                    prev = None
                    for kq in range(npr):
                        cur = []
                        for par in range(2):
                            ptl = pts.tile([128, 2, 512], BF16, tag="pt")
                            emit_scores(ch, kq, par, ptl)
                            cur.append(ptl)
                        if prev is not None:
                            for par in range(2):
                                emit_pv(ppv[par], kq - 1, prev[par])
                        if filler is not None:
                            filler()
                        prev = cur
                    for par in range(2):
                        emit_pv(ppv[par], npr - 1, prev[par])
                    for par in range(2):
                        base = 64 * par
                        drow = stg.tile([1, 512], F32, tag="drow")
                        nc.vector.tensor_copy(drow[0:1, :], ppv[par][64:65, :])
                        drec = stg.tile([1, 512], F32, tag="drec")
                        nc.vector.reciprocal_approx_fast(drec[0:1, :], drow[0:1, :])
                        rB = stg.tile([64, 512], F32, tag="rB")
                        nc.gpsimd.partition_broadcast(rB[:], drec[0:1, :])
                        nc.vector.tensor_tensor(
                            attnT[ch][base : base + 64, t0 : t0 + 512],
                            ppv[par][0:64, :], rB[:], mybir.AluOpType.mult,
                        )

            def wo_groups(b, tcn):
                """Yield 16 closures, each emitting one (tt, dp) output block."""
                for tt in range(4):
                    for dp in range(DIM // 1024):
                        def emit(tt=tt, dp=dp):
                            tl = tcn * 512 + tt * 128
                            ta = b * SEQ + tl
                            wo_ps = psS.tile([128, 2, 512], F32, tag="sq")
                            for j in range(2):
                                dd = dp * 2 + j
                                for ec in range(2):
                                    nc.tensor.matmul(
                                        wo_ps[:, j, :],
                                        attnT[ec][:, tl : tl + 128],
                                        wo_sb[:, ec, dd * 512 : (dd + 1) * 512],
                                        start=(ec == 0), stop=(ec == 1),
                                    )
                            so = stg.tile([128, 1024], F32, tag="so")
                            nc.vector.tensor_copy(
                                so[:], wo_ps[:].rearrange("p a b -> p (a b)")
                            )
                            nc.sync.dma_start(
                                out_p[ta : ta + 128, dp * 1024 : (dp + 1) * 1024], so[:]
                            )
                        yield emit

            def make_filler(groups):
                def filler():
                    n = next(filler.pace)
                    for _ in range(n):
                        g = next(groups, None)
                        if g is not None:
                            g()
                return filler

            def drain(groups):
                for g in groups:
                    g()

            import itertools

            pending_wo = None
            for b in range(BSZ):
                for tcn in range(NT):
                    proj(b, tcn)
                    if causal:
                        nslots = 2 * 2 * (tcn + 1)   # filler calls this chunk
                        if pending_wo is not None:
                            per = max(1, -(-16 // nslots))
                            fill = make_filler(pending_wo)
                            fill.pace = itertools.repeat(per)
                        else:
                            fill = None
                        attention(b, tcn, filler=fill)
                        if pending_wo is not None:
                            drain(pending_wo)
                        pending_wo = wo_groups(b, tcn)
                if not causal:
                    for qj in range(NT):
                        if pending_wo is not None:
                            fill = make_filler(pending_wo)
                            fill.pace = itertools.repeat(1)
                        else:
                            fill = None
                        attention(b, qj, filler=fill)
                        if pending_wo is not None:
                            drain(pending_wo)
                        pending_wo = wo_groups(b, qj)
            drain(pending_wo)

    nc.compile()
    return nc


def _get_nc(causal: bool):
    if causal not in _BUILT:
        _BUILT[causal] = _build(causal)
    return _BUILT[causal]


def kernel(x, freqs_cos, freqs_sin, mask, wq, wk, wv, wo):
    x = np.asarray(x, dtype=np.float32)
    freqs_cos = np.asarray(freqs_cos, dtype=np.float32)
    freqs_sin = np.asarray(freqs_sin, dtype=np.float32)
    mask = np.asarray(mask, dtype=np.float32)
    wq = np.asarray(wq, dtype=np.float32)
    wk = np.asarray(wk, dtype=np.float32)
    wv = np.asarray(wv, dtype=np.float32)
    wo = np.asarray(wo, dtype=np.float32)

    causal = bool(mask.reshape(SEQ, SEQ)[0, -1] < -1e30)
    nc = _get_nc(causal)

    xT = np.ascontiguousarray(x.reshape(BS, DIM).T.astype(NPBF16))

    # RoPE row tables: row p (within 64) = pair (p%64)//2, sign by parity
    j = (np.arange(128) % 64) // 2
    par = np.arange(128) % 2
    cosE = np.ascontiguousarray(freqs_cos[:, j].T)            # [128, SEQ]
    sgn = np.where(par == 1, 1.0, -1.0).astype(np.float32)
    sinE = np.ascontiguousarray(freqs_sin[:, j].T * sgn[:, None])
    pswap = np.zeros((128, 128), dtype=NPBF16)
    idx = np.arange(128)
    pswap[idx, idx ^ 1] = 1.0

    in_maps = []
    for c in range(NCORES):
        wq_c = wq[c * DQ : (c + 1) * DQ]                       # [256, D]
        wk_c = wk[c * HEAD_DIM : (c + 1) * HEAD_DIM]           # [64, D]
        wv_c = wv[c * HEAD_DIM : (c + 1) * HEAD_DIM]
        wf_c = np.ascontiguousarray(
            np.concatenate([wq_c, wk_c, wv_c], axis=0).T.astype(NPBF16))  # [D, 384]
        woT_c = np.ascontiguousarray(
            wo[:, c * DQ : (c + 1) * DQ].T.astype(NPBF16))     # [256, D]
        in_maps.append({
            "xT": xT, "wf": wf_c, "woT": woT_c,
            "cosE": cosE, "sinE": sinE, "pswap": pswap,
        })

    global _last_in_maps
    _last_in_maps = in_maps
    res = run_bass_kernel_spmd(nc, in_maps, core_ids=list(range(NCORES)))
    out = res.results[0]["out_p"].astype(np.float64)
    for c in range(1, NCORES):
        out += res.results[c]["out_p"]
    return out.astype(np.float32).reshape(BSZ, SEQ, DIM)


# revision 17
# speedup vs baseline: 1.1362x; 1.0018x over previous
"""Tensor-parallel GQA attention (RoPE + causal softmax + out-proj) on 8 NeuronCores.

Sharding: heads. Each core owns 4 q-heads + 1 kv-head (wq/wk/wv output rows,
wo input cols). x is replicated; each core computes a partial output
(its heads' contribution through wo) and the host sums the 8 partials.

Per-core dataflow (bf16 matmul operands, fp32 accumulation):
  xT [d, t] --(wfT)--> QT [256, t], KT2 [128(dup), t], VT --(PE transpose)--> Vaug [t, 64|1s]
  RoPE on QT/KT via pair-swap matmul + cos/sin combines.
  S^T [k, q] = KT-chunk.T @ QT-chunk (head parity in 64-partition halves)
  P^T = exp(S^T/8 - 8)  (ACT on 4-bank PSUM spans, causal mask via one affine_select)
  [O^T; denom] = Vaug.T @ P^T  (ones column yields the softmax denominator)
  attn_outT = O^T * 1/denom  (recip_approx + gpsimd partition_broadcast)
  partial = attn_outT.T @ woT  -> DRAM
"""
import numpy as np
import ml_dtypes

import concourse.bass as bass
import concourse.mybir as mybir
from concourse import bacc
from concourse.tile import TileContext
from concourse.bass_utils import run_bass_kernel_spmd
from concourse.masks import make_identity

F32 = mybir.dt.float32
BF16 = mybir.dt.bfloat16
NPBF16 = ml_dtypes.bfloat16

DIM = 2048
N_HEADS = 32
N_KV_HEADS = 8
HEAD_DIM = 64
BSZ = 2
SEQ = 2048
NCORES = 8
HQ = N_HEADS // NCORES          # 4 local q heads
DQ = HQ * HEAD_DIM              # 256
BS = BSZ * SEQ                  # 4096 tokens
NT = SEQ // 512                 # 4 t-chunks (512) per batch
EXP_SHIFT = -8.0
SM_SCALE = 0.125                # 1/sqrt(64)

_BUILT = {}
_last_in_maps = None


def _build(causal: bool):
    nc = bacc.Bacc(None, target_bir_lowering=False, debug=False)

    xT = nc.declare_dram_parameter("xT", [DIM, BS], BF16, isOutput=False)
    wf = nc.declare_dram_parameter("wf", [DIM, DQ + 2 * HEAD_DIM], BF16, isOutput=False)
    woT = nc.declare_dram_parameter("woT", [DQ, DIM], BF16, isOutput=False)
    cosE = nc.declare_dram_parameter("cosE", [128, SEQ], F32, isOutput=False)
    sinE = nc.declare_dram_parameter("sinE", [128, SEQ], F32, isOutput=False)
    pswap = nc.declare_dram_parameter("pswap", [128, 128], BF16, isOutput=False)
    out_p = nc.declare_dram_parameter("out_p", [BS, DIM], F32, isOutput=True)

    with TileContext(nc) as tc:
        with (
            tc.tile_pool(name="wts", bufs=1) as wts,
            tc.tile_pool(name="big", bufs=1) as big,
            tc.tile_pool(name="xs", bufs=2) as xs,
            tc.tile_pool(name="tmp", bufs=3) as tmp,
            tc.tile_pool(name="pts", bufs=4) as pts,
            tc.tile_pool(name="stg", bufs=3) as stg,
            tc.tile_pool(name="psS", bufs=2, space="PSUM") as psS,
            tc.tile_pool(name="psP", bufs=2, space="PSUM") as psP,
            tc.tile_pool(name="psV", bufs=2, space="PSUM") as psV,
        ):
            # ---- resident weights / constants ----
            wf_sb = wts.tile([128, DIM // 128, DQ + 2 * HEAD_DIM], BF16, tag="wf")
            wfr = wf[:].rearrange("(dc p) e -> p dc e", p=128)
            for q4 in range(4):
                nc.sync.dma_start(
                    wf_sb[:, q4 * 4 : (q4 + 1) * 4, :], wfr[:, q4 * 4 : (q4 + 1) * 4, :]
                )
            cos_sb = wts.tile([128, SEQ], F32, tag="cos")
            nc.sync.dma_start(cos_sb[:], cosE[:])
            sin_sb = wts.tile([128, SEQ], F32, tag="sin")
            nc.sync.dma_start(sin_sb[:], sinE[:])
            wo_sb = wts.tile([128, DQ // 128, DIM], BF16, tag="wo")
            nc.sync.dma_start(wo_sb[:], woT[:].rearrange("(ec p) d -> p ec d", p=128))
            psw_sb = wts.tile([128, 128], BF16, tag="psw")
            nc.sync.dma_start(psw_sb[:], pswap[:])
            idt32 = wts.tile([128, 128], F32, tag="idt32")
            make_identity(nc, idt32[:])
            idt = wts.tile([128, 128], BF16, tag="idt")
            nc.vector.tensor_copy(idt[:], idt32[:])

            ones_c = wts.tile([128, 1], F32, tag="ones")
            nc.vector.memset(ones_c[:], 1.0)
            bias_c = wts.tile([128, 1], F32, tag="bias")
            nc.vector.memset(bias_c[:], EXP_SHIFT)

            # ---- per-batch activations (bf16), reused across the 2 batches ----
            QTd = [big.tile([128, SEQ], BF16, tag=f"qtd{i}", name=f"qtd{i}")
                   for i in range(4)]
            KT2 = big.tile([128, SEQ], BF16, tag="kt2")
            Vaug = big.tile([128, SEQ // 128, HEAD_DIM + 1], BF16, tag="vaug")
            attnT = [big.tile([128, SEQ], BF16, tag=f"at{i}", name=f"at{i}")
                     for i in range(2)]
            nc.vector.tensor_copy(
                Vaug[:, :, HEAD_DIM : HEAD_DIM + 1],
                ones_c[:, 0:1, None].to_broadcast((128, SEQ // 128, 1)),
            )

            NKC = DIM // 128  # 16 contraction chunks

            def proj(b, tcn):
                """Project t-chunk (512 tokens), RoPE, fill QT/KT2/Vaug."""
                t0 = b * SEQ + tcn * 512   # global (xT read)
                s0 = tcn * 512             # local within batch
                xtile = xs.tile([128, NKC, 512], BF16, tag="xt")
                xr = xT[:, t0 : t0 + 512].rearrange("(dc p) t -> p dc t", p=128)
                for q4 in range(4):
                    nc.sync.dma_start(
                        xtile[:, q4 * 4 : (q4 + 1) * 4, :], xr[:, q4 * 4 : (q4 + 1) * 4, :]
                    )
                # --- KV chain first (feeds V transposes + K rope early) ---
                pKV = psP.tile([128, 512], F32, tag="pp", name="pkv")
                for dc in range(NKC):
                    nc.tensor.matmul(
                        pKV[:], wf_sb[:, dc, 256:384], xtile[:, dc, :],
                        start=dc == 0, stop=dc == NKC - 1,
                    )
                # K rope
                kraw = tmp.tile([128, 512], BF16, tag="qraw")
                nc.vector.tensor_copy(kraw[0:64, :], pKV[0:64, :])
                # V -> Vaug via PE transpose (V^T at psum partitions 64:128)
                vt_sb = tmp.tile([128, 512], BF16, tag="vt")
                nc.vector.tensor_copy(vt_sb[0:64, :], pKV[64:128, :])
                ksw_ps = psS.tile([128, 2, 512], F32, tag="sq")
                nc.tensor.matmul(
                    ksw_ps[0:64, 0, :], psw_sb[0:64, 0:64], kraw[0:64, :],
                    start=True, stop=True,
                )
                t1 = tmp.tile([128, 512], F32, tag="t1")
                nc.vector.tensor_tensor(
                    t1[0:64, :], kraw[0:64, :], cos_sb[0:64, s0 : s0 + 512],
                    mybir.AluOpType.mult,
                )
                t2 = tmp.tile([128, 512], F32, tag="t2")
                nc.vector.tensor_tensor(
                    t2[0:64, :], ksw_ps[0:64, 0, :], sin_sb[0:64, s0 : s0 + 512],
                    mybir.AluOpType.mult,
                )
                nc.vector.tensor_tensor(
                    KT2[0:64, s0 : s0 + 512], t1[0:64, :], t2[0:64, :],
                    mybir.AluOpType.add,
                )
                # duplicate rope'd K into partitions 64:128 (for per-parity scores)
                nc.sync.dma_start(KT2[64:128, s0 : s0 + 512], KT2[0:64, s0 : s0 + 512])
                for i in range(4):
                    kig = s0 // 128 + i
                    vtp = psS.tile([128, HEAD_DIM], BF16, tag="sq")
                    nc.tensor.transpose(
                        vtp[:, :], vt_sb[0:64, i * 128 : (i + 1) * 128], idt[0:64, 0:64]
                    )
                    nc.vector.tensor_copy(Vaug[:, kig, 0:HEAD_DIM], vtp[:])
                # --- Q chains ---
                for ch in range(2):
                    pQ = psP.tile([128, 512], F32, tag="pp", name="pq")
                    for dc in range(NKC):
                        nc.tensor.matmul(
                            pQ[:], wf_sb[:, dc, ch * 128 : (ch + 1) * 128],
                            xtile[:, dc, :],
                            start=dc == 0, stop=dc == NKC - 1,
                        )
                    qraw = tmp.tile([128, 512], BF16, tag="qraw")
                    nc.vector.tensor_copy(qraw[:], pQ[:])
                    psw_ps = psS.tile([128, 2, 512], F32, tag="sq")
                    nc.tensor.matmul(
                        psw_ps[:, 0, :], psw_sb[:], qraw[:], start=True, stop=True
                    )
                    t1 = tmp.tile([128, 512], F32, tag="t1")
                    nc.vector.tensor_tensor(
                        t1[:], qraw[:], cos_sb[:, s0 : s0 + 512], mybir.AluOpType.mult
                    )
                    t2 = tmp.tile([128, 512], F32, tag="t2")
                    nc.vector.tensor_tensor(
                        t2[:], psw_ps[:, 0, :], sin_sb[:, s0 : s0 + 512],
                        mybir.AluOpType.mult,
                    )
                    qd0, qd1 = QTd[2 * ch], QTd[2 * ch + 1]
                    nc.vector.tensor_tensor(
                        qd1[:, s0 : s0 + 512], t1[:], t2[:], mybir.AluOpType.add
                    )
                    # qd1 now holds [head-even | head-odd]; spread into per-head dups
                    nc.sync.dma_start(qd0[0:64, s0 : s0 + 512], qd1[0:64, s0 : s0 + 512])
                    nc.sync.dma_start(qd0[64:128, s0 : s0 + 512], qd1[0:64, s0 : s0 + 512])
                    nc.sync.dma_start(qd1[0:64, s0 : s0 + 512], qd1[64:128, s0 : s0 + 512])

            def attention(b, qj, filler=None):
                """All 4 local heads for q-chunk qj; filler() emits PE work
                between dependency-stalled attention slots."""
                t0 = qj * 512
                npr = 2 * (qj + 1) if causal else 2 * NT   # k-pairs (2 k-tiles each)
                kmax = 2 * npr

                def emit_scores(ch, kq, par, ptl):
                    h = 2 * ch + par
                    sq_ps = psS.tile([128, 2, 512], F32, tag="sq")
                    for i in range(2):
                        kl = kq * 2 + i
                        nc.tensor.matmul(
                            sq_ps[:, i, :],
                            KT2[64 * i : 64 * i + 64, kl * 128 : (kl + 1) * 128],
                            QTd[h][64 * i : 64 * i + 64, t0 : t0 + 512],
                            start=True, stop=True,
                        )
                    nc.scalar.activation(
                        ptl[:], sq_ps[:], mybir.ActivationFunctionType.Exp,
                        bias=bias_c[:], scale=SM_SCALE,
                    )
                    if causal and kq // 2 == qj:
                        nc.gpsimd.affine_select(
                            out=ptl[:], in_=ptl[:],
                            compare_op=mybir.AluOpType.is_ge,
                            fill=0.0,
                            base=512 * qj - 128 * (2 * kq),
                            pattern=[[-128, 2], [1, 512]],
                            channel_multiplier=-1,
                        )

                def emit_pv(ppv_t, kq, ptl):
                    for i in range(2):
                        kl = kq * 2 + i
                        nc.tensor.matmul(
                            ppv_t[0:65, :], Vaug[:, kl, :], ptl[:, i, :],
                            start=(kl == 0), stop=(kl == kmax - 1),
                            skip_group_check=True,
                        )

                for ch in range(2):           # head pair (2ch, 2ch+1)
                    ppv = [psV.tile([128, 512], F32, tag="pv", name=f"ppv{par}")
                           for par in range(2)]
                    prev = None
                    for kq in range(npr):
                        cur = []
                        for par in range(2):
                            ptl = pts.tile([128, 2, 512], BF16, tag="pt")
                            emit_scores(ch, kq, par, ptl)
                            cur.append(ptl)
                        if prev is not None:
                            for par in range(2):
                                emit_pv(ppv[par], kq - 1, prev[par])
                        if filler is not None:
                            filler()
                        prev = cur
                    for par in range(2):
                        emit_pv(ppv[par], npr - 1, prev[par])
                    for par in range(2):
                        base = 64 * par
                        drow = stg.tile([1, 512], F32, tag="drow")
                        nc.vector.tensor_copy(drow[0:1, :], ppv[par][64:65, :])
                        drec = stg.tile([1, 512], F32, tag="drec")
                        nc.vector.reciprocal_approx_fast(drec[0:1, :], drow[0:1, :])
                        rB = stg.tile([64, 512], F32, tag="rB")
                        nc.gpsimd.partition_broadcast(rB[:], drec[0:1, :])
                        nc.vector.tensor_tensor(
                            attnT[ch][base : base + 64, t0 : t0 + 512],
                            ppv[par][0:64, :], rB[:], mybir.AluOpType.mult,
                        )

            def wo_groups(b, tcn):
                """Yield 16 closures, each emitting one (tt, dp) output block."""
                for tt in range(4):
                    for dp in range(DIM // 1024):
                        def emit(tt=tt, dp=dp):
                            tl = tcn * 512 + tt * 128
                            ta = b * SEQ + tl
                            wo_ps = psS.tile([128, 2, 512], F32, tag="sq")
                            for j in range(2):
                                dd = dp * 2 + j
                                for ec in range(2):
                                    nc.tensor.matmul(
                                        wo_ps[:, j, :],
                                        attnT[ec][:, tl : tl + 128],
                                        wo_sb[:, ec, dd * 512 : (dd + 1) * 512],
                                        start=(ec == 0), stop=(ec == 1),
                                    )
                            so = stg.tile([128, 1024], F32, tag="so")
                            nc.vector.tensor_copy(
                                so[:], wo_ps[:].rearrange("p a b -> p (a b)")
                            )
                            nc.sync.dma_start(
                                out_p[ta : ta + 128, dp * 1024 : (dp + 1) * 1024], so[:]
                            )
                        yield emit

            def make_filler(groups):
                def filler():
                    n = next(filler.pace)
                    for _ in range(n):
                        g = next(groups, None)
                        if g is not None:
                            g()
                return filler

            def drain(groups):
                for g in groups:
                    g()

            import itertools

            pending_wo = None
            for b in range(BSZ):
                for tcn in range(NT):
                    proj(b, tcn)
                    if causal:
                        nslots = 2 * 2 * (tcn + 1)   # filler calls this chunk
                        if pending_wo is not None:
                            per = max(1, -(-16 // nslots))
                            fill = make_filler(pending_wo)
                            fill.pace = itertools.repeat(per)
                        else:
                            fill = None
                        attention(b, tcn, filler=fill)
                        if pending_wo is not None:
                            drain(pending_wo)
                        pending_wo = wo_groups(b, tcn)
                if not causal:
                    for qj in range(NT):
                        if pending_wo is not None:
                            fill = make_filler(pending_wo)
                            fill.pace = itertools.repeat(1)
                        else:
                            fill = None
                        attention(b, qj, filler=fill)
                        if pending_wo is not None:
                            drain(pending_wo)
                        pending_wo = wo_groups(b, qj)
            drain(pending_wo)

    nc.compile()
    return nc


def _get_nc(causal: bool):
    if causal not in _BUILT:
        _BUILT[causal] = _build(causal)
    return _BUILT[causal]


def kernel(x, freqs_cos, freqs_sin, mask, wq, wk, wv, wo):
    x = np.asarray(x, dtype=np.float32)
    freqs_cos = np.asarray(freqs_cos, dtype=np.float32)
    freqs_sin = np.asarray(freqs_sin, dtype=np.float32)
    mask = np.asarray(mask, dtype=np.float32)
    wq = np.asarray(wq, dtype=np.float32)
    wk = np.asarray(wk, dtype=np.float32)
    wv = np.asarray(wv, dtype=np.float32)
    wo = np.asarray(wo, dtype=np.float32)

    causal = bool(mask.reshape(SEQ, SEQ)[0, -1] < -1e30)
    nc = _get_nc(causal)

    xT = np.ascontiguousarray(x.reshape(BS, DIM).T.astype(NPBF16))

    # RoPE row tables: row p (within 64) = pair (p%64)//2, sign by parity
    j = (np.arange(128) % 64) // 2
    par = np.arange(128) % 2
    cosE = np.ascontiguousarray(freqs_cos[:, j].T)            # [128, SEQ]
    sgn = np.where(par == 1, 1.0, -1.0).astype(np.float32)
    sinE = np.ascontiguousarray(freqs_sin[:, j].T * sgn[:, None])
    pswap = np.zeros((128, 128), dtype=NPBF16)
    idx = np.arange(128)
    pswap[idx, idx ^ 1] = 1.0

    in_maps = []
    for c in range(NCORES):
        wq_c = wq[c * DQ : (c + 1) * DQ]                       # [256, D]
        wk_c = wk[c * HEAD_DIM : (c + 1) * HEAD_DIM]           # [64, D]
        wv_c = wv[c * HEAD_DIM : (c + 1) * HEAD_DIM]
        wf_c = np.ascontiguousarray(
            np.concatenate([wq_c, wk_c, wv_c], axis=0).T.astype(NPBF16))  # [D, 384]
        woT_c = np.ascontiguousarray(
            wo[:, c * DQ : (c + 1) * DQ].T.astype(NPBF16))     # [256, D]
        in_maps.append({
            "xT": xT, "wf": wf_c, "woT": woT_c,
            "cosE": cosE, "sinE": sinE, "pswap": pswap,
        })

    global _last_in_maps
    _last_in_maps = in_maps
    res = run_bass_kernel_spmd(nc, in_maps, core_ids=list(range(NCORES)))
    out = res.results[0]["out_p"].astype(np.float64)
    for c in range(1, NCORES):
        out += res.results[c]["out_p"]
    return out.astype(np.float32).reshape(BSZ, SEQ, DIM)
